# revision 1
# baseline (speedup 1.0000x reference)
"""MiMoV2 MoE gate (moe_routing) on 8 Trainium2 NeuronCores.

Strategy (v2):
  - Shard tokens (bsz*seq = 16384) across 8 cores, 2048 tokens each;
    replicate the [256, 4096] gate weight + bias.
  - Gating GEMM with W stationary and tokens moving (N=512), output
    [expert, token] in PSUM. Precision via fp16 main + ONE stacked
    fp8e4m3 DoubleRow correction pass:
      logits*2^17 = (x1*2^8)(W1*2^9)            [fp16, exact products]
                  + (dx*2^12)(W1*2^5)           [fp8 DR, chunk-paired]
                  + (x*2^-1)(dW*2^18)           [fp8 DR, chunk-paired]
    where x1 = fp16(x), dx = x - x1, W1 = fp16(W), dW = W - W1. All
    three pieces share one PSUM accumulation (scales match at 2^17), so
    no combine op is needed; the 2^-17 descale rides the psum->sbuf
    copy. Residual logit error ~1e-5 abs (vs fp16-single's 3.8e-4 which
    flips too many expert choices).
  - DoubleRow packs 2 contraction chunks per matmul (2 rows/PE cell),
    halving correction matmul time; its 256-col LDWEIGHTS is amortized
    by streaming 2 token-blocks per weight load.
  - PE transpose (identity matmul) returns logits to [token, expert];
    sigmoid+routing identical in spirit to v1: per-group top-2 via
    segmented reduce_max + match_replace; top-4 groups via max8
    threshold; exact-passthrough masking; top-8 via max8 + max_index;
    weights via masked max8 over raw scores + 8x8 index-match permute.

Inputs (full):  hidden_states [4,4096,4096] f32, weight [256,4096] f32,
                e_score_correction_bias [256] f32
Output (full):  (topk_idx [16384,8] int32, topk_weight [16384,8] f32)
"""

import numpy as np
import ml_dtypes

import concourse.tile as tile
from concourse import bacc, mybir
from concourse.bass_utils import run_bass_kernel_spmd

# problem shape (hardcoded per contract)
T_FULL = 16384
H = 4096
E = 256
G = 8
GS = E // G           # 32
TOPK = 8
SCALING = 2.5

N_CORES = 8
T_CORE = T_FULL // N_CORES    # 2048
NCH = H // 128                # 32 contraction chunks
NQ = NCH // 2                 # 16 chunk-pairs for DoubleRow
TB = 512                      # token block (psum bank = 512 f32)
NB = T_CORE // TB             # 4 blocks
NSUB = TB // 128              # 4 token subtiles per block

SC_MAIN = 2.0 ** 17           # psum scale
S_X1 = 2.0 ** 8               # x1 pre-scale (x1*W1 -> 2^17)
S_W1 = 2.0 ** 9
S_DX = 2.0 ** 12              # dx pre-scale (dx*W1 -> 2^17)
S_W1_8 = 2.0 ** 5
S_XC = 2.0 ** -1              # coarse-x pre-scale (x*dW -> 2^17)
S_DW = 2.0 ** 18

_BUILT = None


def _build():
    f32 = mybir.dt.float32
    f16 = mybir.dt.float16
    f8 = mybir.dt.float8e4
    u32 = mybir.dt.uint32
    AF = mybir.ActivationFunctionType
    OP = mybir.AluOpType
    AX = mybir.AxisListType
    DR = mybir.MatmulPerfMode.DoubleRow

    nc = bacc.Bacc("TRN2", target_bir_lowering=False, debug=False)

    # x arrays, contraction on partitions, block/chunk/token free layout
    x1 = nc.dram_tensor("x1", [NB, 128, NCH, TB], f16, kind="ExternalInput").ap()
    dx8 = nc.dram_tensor("dx8", [NB, 128, NCH, TB], f8, kind="ExternalInput").ap()
    xc8 = nc.dram_tensor("xc8", [NB, 128, NCH, TB], f8, kind="ExternalInput").ap()
    # W arrays: [128, chunk, ehalf, 128e]
    w1 = nc.dram_tensor("w1", [128, NCH, 2, 128], f16, kind="ExternalInput").ap()
    w18 = nc.dram_tensor("w18", [128, NCH, 2, 128], f8, kind="ExternalInput").ap()
    dw8 = nc.dram_tensor("dw8", [128, NCH, 2, 128], f8, kind="ExternalInput").ap()
    bias_rep = nc.dram_tensor("bias_rep", [128, E], f32, kind="ExternalInput").ap()
    id_in = nc.dram_tensor("id_in", [128, 128], f32, kind="ExternalInput").ap()

    idx_out = nc.dram_tensor("idx_out", [T_CORE, TOPK], u32, kind="ExternalOutput").ap()
    w_out = nc.dram_tensor("w_out", [T_CORE, TOPK], f32, kind="ExternalOutput").ap()

    with tile.TileContext(nc) as tc:
        with tc.tile_pool(name="const", bufs=1) as cpool, \
             tc.tile_pool(name="xin", bufs=1) as xpool, \
             tc.tile_pool(name="comb", bufs=3) as kpool, \
             tc.tile_pool(name="mid", bufs=4) as mpool, \
             tc.tile_pool(name="small", bufs=8) as spool, \
             tc.tile_pool(name="pacc", bufs=2, space="PSUM") as papool, \
             tc.tile_pool(name="warm", bufs=1, space="PSUM") as wpool, \
             tc.tile_pool(name="ptr", bufs=3, space="PSUM") as ptpool:

            # constants: W tiles (chunk-quartered DMA so first MMs start
            # early), bias, identity
            W1t = cpool.tile([128, NCH, 2, 128], f16, tag="W1t")
            W18t = cpool.tile([128, NCH, 2, 128], f8, tag="W18t")
            dW8t = cpool.tile([128, NCH, 2, 128], f8, tag="dW8t")
            BR = cpool.tile([128, E], f32, tag="BR")
            IDT = cpool.tile([128, 128], f32, tag="IDT")
            # W/bias/ident on the Scalar engine's DMA queue so they stream
            # in parallel with the x loads on the Sync queue
            QC = NCH // 4
            for q in range(4):
                sl = slice(q * QC, (q + 1) * QC)
                nc.scalar.dma_start(W1t[:, sl, :, :], w1[:, sl, :, :])
                nc.scalar.dma_start(W18t[:, sl, :, :], w18[:, sl, :, :])
                nc.scalar.dma_start(dW8t[:, sl, :, :], dw8[:, sl, :, :])
                if q == 0:
                    nc.scalar.dma_start(IDT[:], id_in)
                    nc.scalar.dma_start(BR[:], bias_rep)

            # HAM warm-up: ~64 dummy matmuls on the first W quarter (lands
            # within ~2us) keep the PE busy through the x-DMA latency so the
            # 2.4GHz clock gate opens before the real MM stream begins.
            # (Transpose-mode does not count as PE-busy for HAM.)
            pw = wpool.tile([128, TB], f32, tag="pw")
            wrhs = W1t[:, 0:4, 0, :]
            for r in range(64):
                nc.tensor.matmul(pw[:], W1t[:, 0, 0, :], wrhs,
                                 start=(r == 0), stop=(r == 63))

            BLOCKS = [(0, 0, TB, 0), (1, 0, TB, TB), (2, 0, TB, 2 * TB),
                      (3, 0, TB // 2, 3 * TB), (3, TB // 2, TB // 2, 3 * TB + TB // 2)]
            for bi, (bsrc, boff, tbs, tbase) in enumerate(BLOCKS):
                b = bi
                xt1 = xpool.tile([128, NCH, TB], f16, tag=f"x1_{b % 2}", name=f"xt1_{b % 2}")
                xd8 = xpool.tile([128, NCH, TB], f8, tag=f"dx_{b % 2}", name=f"xd8_{b % 2}")
                xc8t = xpool.tile([128, NCH, TB], f8, tag=f"xc_{b % 2}", name=f"xc8t_{b % 2}")
                tsl = slice(boff, boff + tbs)
                nsp = 4 if b == 0 else 2
                PC = NCH // nsp
                for hf in range(nsp):
                    sl = slice(hf * PC, (hf + 1) * PC)
                    nc.sync.dma_start(xt1[:, sl, 0:tbs], x1[bsrc][:, sl, tsl])
                    nc.sync.dma_start(xd8[:, sl, 0:tbs], dx8[bsrc][:, sl, tsl])
                    nc.sync.dma_start(xc8t[:, sl, 0:tbs], xc8[bsrc][:, sl, tsl])

                # 1:1 interleave of main fp16 MMs (N=512 stream, 213ns) and
                # fp8 DoubleRow corr MMs: each DR LDWEIGHTS (256 cols,
                # ~213ns) prefetches into the background weight buffer
                # during the preceding main MM's stream.
                ps = {}
                for h in range(2):
                    ps[h] = papool.tile([128, TB], f32, tag=f"ps{h}", name=f"ps_{h}")
                    for g in range(NCH):
                        nc.tensor.matmul(ps[h][:, 0:tbs], W1t[:, g, h, :],
                                         xt1[:, g, 0:tbs],
                                         start=(g == 0), stop=False)
                        q = g // 2
                        if g % 2 == 0:
                            nc.tensor.matmul(ps[h][:, 0:tbs],
                                             W18t[:, 2 * q:2 * q + 2, h, :],
                                             xd8[:, 2 * q:2 * q + 2, 0:tbs],
                                             perf_mode=DR, start=False, stop=False)
                        else:
                            nc.tensor.matmul(ps[h][:, 0:tbs],
                                             dW8t[:, 2 * q:2 * q + 2, h, :],
                                             xc8t[:, 2 * q:2 * q + 2, 0:tbs],
                                             perf_mode=DR, start=False,
                                             stop=(g == NCH - 1))

                # ---- routing, breadth-first across the 4 subtiles so the
                # DVE queue never head-of-line blocks on one chain ----
                cb = {}
                for h in range(2):
                    cb[h] = kpool.tile([128, TB], f32, tag=f"cb{h}", name=f"cb_{h}")
                    nc.scalar.activation(cb[h][:, 0:tbs], ps[h][:, 0:tbs], AF.Copy,
                                         scale=1.0 / SC_MAIN)
                NSUBB = tbs // 128
                T = {}
                for g in range(NSUBB):
                    pt = ptpool.tile([128, E], f32, tag="pt", name="pt")
                    for h in range(2):
                        nc.tensor.transpose(pt[:, h * 128:(h + 1) * 128],
                                            cb[h][:, g * 128:(g + 1) * 128],
                                            IDT[:])
                    s_raw = mpool.tile([128, E], f32, tag="s_raw", name="s_raw")
                    nc.scalar.activation(s_raw[:], pt[:], AF.Sigmoid)
                    s_choice = mpool.tile([128, E], f32, tag="s_choice", name="s_choice")
                    nc.vector.tensor_add(s_choice[:], s_raw[:], BR[:])
                    T[g] = {"s_raw": s_raw, "s_choice": s_choice}
                for g in range(NSUBB):
                    m1 = spool.tile([128, G], f32, tag="m1", name="m1")
                    nc.vector.reduce_max(
                        m1[:], T[g]["s_choice"][:].rearrange("p (g s) -> p g s", g=G),
                        axis=AX.X)
                    T[g]["m1"] = m1
                for g in range(NSUBB):
                    repl = mpool.tile([128, E], f32, tag="repl", name="repl")
                    nc.vector.match_replace(repl[:], T[g]["m1"][:],
                                            T[g]["s_choice"][:], -1e30)
                    T[g]["repl"] = repl
                for g in range(NSUBB):
                    m2 = spool.tile([128, G], f32, tag="m2", name="m2")
                    nc.vector.reduce_max(
                        m2[:], T[g]["repl"][:].rearrange("p (g s) -> p g s", g=G),
                        axis=AX.X)
                    T[g]["m2"] = m2
                for g in range(NSUBB):
                    gsum = spool.tile([128, G], f32, tag="gsum", name="gsum")
                    nc.vector.tensor_add(gsum[:], T[g]["m1"][:], T[g]["m2"][:])
                    T[g]["gsum"] = gsum
                for g in range(NSUBB):
                    gs8 = spool.tile([128, 8], f32, tag="gs8", name="gs8")
                    nc.vector.max(gs8[:], T[g]["gsum"][:])
                    T[g]["gs8"] = gs8
                for g in range(NSUBB):
                    pen = spool.tile([128, G], f32, tag="pen", name="pen")
                    nc.vector.tensor_scalar(pen[:], T[g]["gsum"][:],
                                            T[g]["gs8"][:, 3:4],
                                            -1e30, op0=OP.is_lt, op1=OP.mult)
                    T[g]["pen"] = pen
                for g in range(NSUBB):
                    s_mask = mpool.tile([128, E], f32, tag="s_mask", name="s_mask")
                    pen_b = T[g]["pen"][:].unsqueeze(2).broadcast_to([128, G, GS])
                    nc.vector.tensor_tensor(
                        s_mask[:].rearrange("p (g s) -> p g s", g=G),
                        T[g]["s_choice"][:].rearrange("p (g s) -> p g s", g=G),
                        pen_b, op=OP.add)
                    T[g]["s_mask"] = s_mask
                for g in range(NSUBB):
                    v8 = spool.tile([128, 8], f32, tag="v8", name="v8")
                    nc.vector.max(v8[:], T[g]["s_mask"][:])
                    T[g]["v8"] = v8
                for g in range(NSUBB):
                    i8 = spool.tile([128, 8], u32, tag="i8", name="i8")
                    nc.vector.max_index(i8[:], T[g]["v8"][:], T[g]["s_mask"][:])
                    tok0 = tbase + g * 128
                    nc.gpsimd.dma_start(idx_out[tok0:tok0 + 128, :], i8[:])
                    T[g]["i8"] = i8
                for g in range(NSUBB):
                    r_sel = mpool.tile([128, E], f32, tag="r_sel", name="r_sel")
                    nc.vector.scalar_tensor_tensor(
                        r_sel[:], in0=T[g]["s_mask"][:], scalar=T[g]["v8"][:, 7:8],
                        in1=T[g]["s_raw"][:], op0=OP.is_ge, op1=OP.mult)
                    T[g]["r_sel"] = r_sel
                for g in range(NSUBB):
                    w8d = spool.tile([128, 8], f32, tag="w8d", name="w8d")
                    nc.vector.max(w8d[:], T[g]["r_sel"][:])
                    T[g]["w8d"] = w8d
                for g in range(NSUBB):
                    ri8 = spool.tile([128, 8], u32, tag="ri8", name="ri8")
                    nc.vector.max_index(ri8[:], T[g]["w8d"][:], T[g]["r_sel"][:])
                    T[g]["ri8"] = ri8
                for g in range(NSUBB):
                    eq64 = spool.tile([128, 8, 8], f32, tag="eq64", name="eq64")
                    i8_b = T[g]["i8"][:].unsqueeze(2).broadcast_to([128, 8, 8])
                    ri8_b = T[g]["ri8"][:].unsqueeze(1).broadcast_to([128, 8, 8])
                    nc.vector.tensor_tensor(eq64[:], i8_b, ri8_b, op=OP.is_equal)
                    T[g]["eq64"] = eq64
                for g in range(NSUBB):
                    w64 = spool.tile([128, 8, 8], f32, tag="w64", name="w64")
                    w8d_b = T[g]["w8d"][:].unsqueeze(1).broadcast_to([128, 8, 8])
                    nc.vector.tensor_tensor(w64[:], T[g]["eq64"][:], w8d_b,
                                            op=OP.mult)
                    T[g]["w64"] = w64
                for g in range(NSUBB):
                    w8p = spool.tile([128, 8], f32, tag="w8p", name="w8p")
                    nc.vector.reduce_sum(w8p[:], T[g]["w64"][:], axis=AX.X)
                    T[g]["w8p"] = w8p
                for g in range(NSUBB):
                    sum8 = spool.tile([128, 1], f32, tag="sum8", name="sum8")
                    nc.vector.reduce_sum(sum8[:], T[g]["w8d"][:], axis=AX.X)
                    T[g]["sum8"] = sum8
                for g in range(NSUBB):
                    rcp = spool.tile([128, 1], f32, tag="rcp", name="rcp")
                    nc.vector.reciprocal(rcp[:], T[g]["sum8"][:])
                    T[g]["rcp"] = rcp
                for g in range(NSUBB):
                    wf = spool.tile([128, 8], f32, tag="wf", name="wf")
                    nc.vector.tensor_scalar(wf[:], T[g]["w8p"][:],
                                            T[g]["rcp"][:, 0:1],
                                            SCALING, op0=OP.mult, op1=OP.mult)
                    tok0 = tbase + g * 128
                    nc.gpsimd.dma_start(w_out[tok0:tok0 + 128, :], wf[:])

    nc.compile()
    return nc


def _get_built():
    global _BUILT
    if _BUILT is None:
        _BUILT = _build()
    return _BUILT


def _part(a, inner):
    # [H, inner] -> [128, NCH, inner] with element (p, c, i) = a[c*128+p, i]
    return np.ascontiguousarray(a.reshape(NCH, 128, inner).transpose(1, 0, 2))


def _prep_in_maps(hidden_states, weight, e_score_correction_bias):
    f8 = ml_dtypes.float8_e4m3
    x = np.asarray(hidden_states, dtype=np.float32).reshape(T_FULL, H)
    xT = np.ascontiguousarray(x.T)                      # [H, T]
    x1f = xT.astype(np.float16)
    dx = xT - x1f.astype(np.float32)

    x1s = (x1f.astype(np.float32) * S_X1).astype(np.float16)   # exact scale
    dx8f = (dx * S_DX).astype(f8)
    xc8f = (xT * S_XC).astype(f8)

    W = np.asarray(weight, dtype=np.float32)
    Wt = np.ascontiguousarray(W.T)                      # [H, E]
    W1f = Wt.astype(np.float16)
    dW = Wt - W1f.astype(np.float32)
    w1h = _part((W1f.astype(np.float32) * S_W1).astype(np.float16), E)
    w18h = _part((W1f.astype(np.float32) * S_W1_8).astype(f8), E)
    dw8h = _part((dW * S_DW).astype(f8), E)
    w1h = w1h.reshape(128, NCH, 2, 128)
    w18h = w18h.reshape(128, NCH, 2, 128)
    dw8h = dw8h.reshape(128, NCH, 2, 128)

    b = np.asarray(e_score_correction_bias, dtype=np.float32)
    bias_rep = np.ascontiguousarray(np.tile(b[None, :], (128, 1)))
    ident = np.eye(128, dtype=np.float32)

    def blocks(a):
        # [128, NCH, T_CORE] -> [NB, 128, NCH, TB]
        v = a.reshape(128, NCH, NB, TB)
        return np.ascontiguousarray(v.transpose(2, 0, 1, 3))

    in_maps = []
    for c in range(N_CORES):
        sl = slice(c * T_CORE, (c + 1) * T_CORE)
        in_maps.append({
            "x1": blocks(_part(x1s[:, sl], T_CORE)),
            "dx8": blocks(_part(dx8f[:, sl], T_CORE)),
            "xc8": blocks(_part(xc8f[:, sl], T_CORE)),
            "w1": w1h, "w18": w18h, "dw8": dw8h,
            "bias_rep": bias_rep, "id_in": ident,
        })
    return in_maps


def kernel(hidden_states: np.ndarray, weight: np.ndarray,
           e_score_correction_bias: np.ndarray):
    in_maps = _prep_in_maps(hidden_states, weight, e_score_correction_bias)
    nc = _get_built()
    res = run_bass_kernel_spmd(nc, in_maps, list(range(N_CORES)))

    idx = np.concatenate([r["idx_out"] for r in res.results], axis=0).astype(np.int32)
    w = np.concatenate([r["w_out"] for r in res.results], axis=0).astype(np.float32)
    return idx, w



# revision 2
# speedup vs baseline: 1.0557x; 1.0557x over previous
"""MiMoV2 MoE gate (moe_routing) on 8 Trainium2 NeuronCores.

Strategy (v3):
  - Shard tokens (bsz*seq = 16384) across 8 cores, 2048 tokens each;
    replicate the [256, 4096] gate weight + bias.
  - Gating GEMM with W stationary and tokens moving (N=512), output
    [expert, token] in PSUM. Precision via fp16 main + ONE stacked
    fp8e4m3 DoubleRow correction pass:
      logits*2^17 = (x1*2^8)(W1*2^9)            [fp16, exact products]
                  + (dx*2^12)(W1*2^5)           [fp8 DR, chunk-paired]
                  + (x*2^-1)(dW*2^18)           [fp8 DR, chunk-paired]
    All three pieces share one PSUM accumulation; 2^-17 descale rides
    the psum->sbuf copy. Residual logit sigma ~1.3e-5.
  - v3 vs v2: HAM warmup shrunk from 64xN512 MMs (15.7us) to 8 fp32
    N=128 MMs on the identity tile (~3.4us, exactly the HAM window);
    x / W DRAM layouts flattened so every DMA is 128 fully-contiguous
    per-partition lines (descriptor-gen was 1.2us/DMA, now ~0.2);
    block 0 x1 delivered in 8ths so the real MM stream starts ~4us in;
    block 3 loaded contiguously (512 tokens) and MM'd as two 256-token
    halves reading SBUF slices.
  - Routing identical to v2: per-group top-2 via segmented reduce_max +
    match_replace; top-4 groups via max8 threshold; exact-passthrough
    masking; top-8 via max8 + max_index; weights via masked max8 over
    raw scores + 8x8 index-match permute.

Inputs (full):  hidden_states [4,4096,4096] f32, weight [256,4096] f32,
                e_score_correction_bias [256] f32
Output (full):  (topk_idx [16384,8] int32, topk_weight [16384,8] f32)
"""

import numpy as np
import ml_dtypes

import concourse.tile as tile
from concourse import bacc, mybir
from concourse.bass_utils import run_bass_kernel_spmd

# problem shape (hardcoded per contract)
T_FULL = 16384
H = 4096
E = 256
G = 8
GS = E // G           # 32
TOPK = 8
SCALING = 2.5

N_CORES = 8
T_CORE = T_FULL // N_CORES    # 2048
NCH = H // 128                # 32 contraction chunks
NQ = NCH // 2                 # 16 chunk-pairs for DoubleRow
TB = 512                      # token block (psum bank = 512 f32)
NB = T_CORE // TB             # 4 blocks
XF = NCH * TB                 # flat free size of one x block per partition
WF = NCH * 2 * 128            # flat free size of W per partition

SC_MAIN = 2.0 ** 17           # psum scale
S_X1 = 2.0 ** 8               # x1 pre-scale (x1*W1 -> 2^17)
S_W1 = 2.0 ** 9
S_DX = 2.0 ** 12              # dx pre-scale (dx*W1 -> 2^17)
S_W1_8 = 2.0 ** 5
S_XC = 2.0 ** -1              # coarse-x pre-scale (x*dW -> 2^17)
S_DW = 2.0 ** 18

_BUILT = None


def _build():
    f32 = mybir.dt.float32
    f16 = mybir.dt.float16
    f8 = mybir.dt.float8e4
    u32 = mybir.dt.uint32
    AF = mybir.ActivationFunctionType
    OP = mybir.AluOpType
    AX = mybir.AxisListType
    DR = mybir.MatmulPerfMode.DoubleRow

    nc = bacc.Bacc("TRN2", target_bir_lowering=False, debug=False)

    # x arrays: flat per-partition layout, elem (b, p, c*TB+t) = x[c*128+p,
    # b*TB+t]; every DMA slice below is contiguous per partition.
    x1 = nc.dram_tensor("x1", [NB, 128, XF], f16, kind="ExternalInput").ap()
    dx8 = nc.dram_tensor("dx8", [NB, 128, XF], f8, kind="ExternalInput").ap()
    xc8 = nc.dram_tensor("xc8", [NB, 128, XF], f8, kind="ExternalInput").ap()
    # W arrays: flat [128, chunk*ehalf*128e]
    w1 = nc.dram_tensor("w1", [128, WF], f16, kind="ExternalInput").ap()
    w18 = nc.dram_tensor("w18", [128, WF], f8, kind="ExternalInput").ap()
    dw8 = nc.dram_tensor("dw8", [128, WF], f8, kind="ExternalInput").ap()
    bias_rep = nc.dram_tensor("bias_rep", [128, E], f32, kind="ExternalInput").ap()
    id_in = nc.dram_tensor("id_in", [128, 128], f32, kind="ExternalInput").ap()

    idx_out = nc.dram_tensor("idx_out", [T_CORE, TOPK], u32, kind="ExternalOutput").ap()
    w_out = nc.dram_tensor("w_out", [T_CORE, TOPK], f32, kind="ExternalOutput").ap()

    with tile.TileContext(nc) as tc:
        with tc.tile_pool(name="const", bufs=1) as cpool, \
             tc.tile_pool(name="xin", bufs=1) as xpool, \
             tc.tile_pool(name="comb", bufs=3) as kpool, \
             tc.tile_pool(name="mid", bufs=4) as mpool, \
             tc.tile_pool(name="small", bufs=8) as spool, \
             tc.tile_pool(name="pacc", bufs=2, space="PSUM") as papool, \
             tc.tile_pool(name="warm", bufs=1, space="PSUM") as wpool, \
             tc.tile_pool(name="ptr", bufs=3, space="PSUM") as ptpool:

            # constants: identity FIRST (warmup dep), then W quarters, bias
            W1t = cpool.tile([128, WF], f16, tag="W1t")
            W18t = cpool.tile([128, WF], f8, tag="W18t")
            dW8t = cpool.tile([128, WF], f8, tag="dW8t")
            BR = cpool.tile([128, E], f32, tag="BR")
            IDT = cpool.tile([128, 128], f32, tag="IDT")
            nc.scalar.dma_start(IDT[:], id_in)
            QW = WF // 4
            for q in range(4):
                sl = slice(q * QW, (q + 1) * QW)
                nc.scalar.dma_start(W1t[:, sl], w1[:, sl])
                nc.scalar.dma_start(W18t[:, sl], w18[:, sl])
                nc.scalar.dma_start(dW8t[:, sl], dw8[:, sl])
                if q == 0:
                    nc.scalar.dma_start(BR[:], bias_rep)

            # 4D views for matmul operands: [128, chunk, ehalf, 128e]
            W1v = W1t[:].rearrange("p (c h e) -> p c h e", c=NCH, h=2)
            W18v = W18t[:].rearrange("p (c h e) -> p c h e", c=NCH, h=2)
            dW8v = dW8t[:].rearrange("p (c h e) -> p c h e", c=NCH, h=2)

            # HAM warmup: PE must be busy ~3.4us (one activity window) for
            # the clock gate to open; 8 fp32 N=128 MMs on the identity tile
            # (~427ns each cold) cover it while the first x DMAs land.
            pw = wpool.tile([128, 128], f32, tag="pw")
            for r in range(8):
                nc.tensor.matmul(pw[:], IDT[:], IDT[:],
                                 start=(r == 0), stop=(r == 7))

            # x tiles: flat [128, XF]; per-block double buffering by tag
            xt1 = {}
            xd8 = {}
            xc8t = {}
            xv1 = {}
            xvd = {}
            xvc = {}
            for par in range(2):
                xt1[par] = xpool.tile([128, XF], f16, tag=f"x1_{par}",
                                      name=f"xt1_{par}")
                xd8[par] = xpool.tile([128, XF], f8, tag=f"dx_{par}",
                                      name=f"xd8_{par}")
                xc8t[par] = xpool.tile([128, XF], f8, tag=f"xc_{par}",
                                       name=f"xc8t_{par}")
                xv1[par] = xt1[par][:].rearrange("p (c t) -> p c t", c=NCH)
                xvd[par] = xd8[par][:].rearrange("p (c t) -> p c t", c=NCH)
                xvc[par] = xc8t[par][:].rearrange("p (c t) -> p c t", c=NCH)

            def load_block(b):
                par = b % 2
                if b == 0:
                    # 8th-granular x1 + quarter dx/xc, interleaved so the
                    # main/DR MM stream can start ~4us in
                    E8 = XF // 8
                    Q4 = XF // 4
                    order = [("x1", 0), ("dx", 0), ("xc", 0), ("x1", 1),
                             ("x1", 2), ("dx", 1), ("xc", 1), ("x1", 3),
                             ("x1", 4), ("dx", 2), ("xc", 2), ("x1", 5),
                             ("x1", 6), ("dx", 3), ("xc", 3), ("x1", 7)]
                    for kind, i in order:
                        if kind == "x1":
                            sl = slice(i * E8, (i + 1) * E8)
                            nc.sync.dma_start(xt1[par][:, sl], x1[b][:, sl])
                        elif kind == "dx":
                            sl = slice(i * Q4, (i + 1) * Q4)
                            nc.sync.dma_start(xd8[par][:, sl], dx8[b][:, sl])
                        else:
                            sl = slice(i * Q4, (i + 1) * Q4)
                            nc.sync.dma_start(xc8t[par][:, sl], xc8[b][:, sl])
                else:
                    HF = XF // 2
                    for i in range(2):
                        sl = slice(i * HF, (i + 1) * HF)
                        nc.sync.dma_start(xt1[par][:, sl], x1[b][:, sl])
                        nc.sync.dma_start(xd8[par][:, sl], dx8[b][:, sl])
                        nc.sync.dma_start(xc8t[par][:, sl], xc8[b][:, sl])

            load_block(0)
            load_block(1)

            # (dram_block, token_offset, tokens, out_token_base)
            BLOCKS = [(0, 0, TB, 0), (1, 0, TB, TB), (2, 0, TB, 2 * TB),
                      (3, 0, TB // 2, 3 * TB),
                      (3, TB // 2, TB // 2, 3 * TB + TB // 2)]
            for bi, (bsrc, boff, tbs, tbase) in enumerate(BLOCKS):
                par = bsrc % 2
                if bi in (1, 2):
                    load_block(bsrc + 1)   # prefetch next dram block
                tsl = slice(boff, boff + tbs)

                # 1:1 interleave of main fp16 MMs (N=tbs stream) and fp8
                # DoubleRow corr MMs: each DR LDWEIGHTS (256 cols) prefetches
                # into the background weight buffer during the preceding main
                # MM's stream.
                ps = {}
                for h in range(2):
                    ps[h] = papool.tile([128, TB], f32, tag=f"ps{h}",
                                        name=f"ps_{h}")
                    for g in range(NCH):
                        nc.tensor.matmul(ps[h][:, 0:tbs], W1v[:, g, h, :],
                                         xv1[par][:, g, tsl],
                                         start=(g == 0), stop=False)
                        q = g // 2
                        if g % 2 == 0:
                            nc.tensor.matmul(ps[h][:, 0:tbs],
                                             W18v[:, 2 * q:2 * q + 2, h, :],
                                             xvd[par][:, 2 * q:2 * q + 2, tsl],
                                             perf_mode=DR, start=False,
                                             stop=False)
                        else:
                            nc.tensor.matmul(ps[h][:, 0:tbs],
                                             dW8v[:, 2 * q:2 * q + 2, h, :],
                                             xvc[par][:, 2 * q:2 * q + 2, tsl],
                                             perf_mode=DR, start=False,
                                             stop=(g == NCH - 1))

                # ---- routing, breadth-first across the subtiles so the
                # DVE queue never head-of-line blocks on one chain ----
                cb = {}
                for h in range(2):
                    cb[h] = kpool.tile([128, TB], f32, tag=f"cb{h}",
                                       name=f"cb_{h}")
                    nc.scalar.activation(cb[h][:, 0:tbs], ps[h][:, 0:tbs],
                                         AF.Copy, scale=1.0 / SC_MAIN)
                NSUBB = tbs // 128
                T = {}
                for g in range(NSUBB):
                    pt = ptpool.tile([128, E], f32, tag="pt", name="pt")
                    for h in range(2):
                        nc.tensor.transpose(pt[:, h * 128:(h + 1) * 128],
                                            cb[h][:, g * 128:(g + 1) * 128],
                                            IDT[:])
                    s_raw = mpool.tile([128, E], f32, tag="s_raw", name="s_raw")
                    nc.scalar.activation(s_raw[:], pt[:], AF.Sigmoid)
                    s_choice = mpool.tile([128, E], f32, tag="s_choice",
                                          name="s_choice")
                    nc.vector.tensor_add(s_choice[:], s_raw[:], BR[:])
                    T[g] = {"s_raw": s_raw, "s_choice": s_choice}
                for g in range(NSUBB):
                    m1 = spool.tile([128, G], f32, tag="m1", name="m1")
                    nc.vector.reduce_max(
                        m1[:], T[g]["s_choice"][:].rearrange("p (g s) -> p g s", g=G),
                        axis=AX.X)
                    T[g]["m1"] = m1
                for g in range(NSUBB):
                    repl = mpool.tile([128, E], f32, tag="repl", name="repl")
                    nc.vector.match_replace(repl[:], T[g]["m1"][:],
                                            T[g]["s_choice"][:], -1e30)
                    T[g]["repl"] = repl
                for g in range(NSUBB):
                    m2 = spool.tile([128, G], f32, tag="m2", name="m2")
                    nc.vector.reduce_max(
                        m2[:], T[g]["repl"][:].rearrange("p (g s) -> p g s", g=G),
                        axis=AX.X)
                    T[g]["m2"] = m2
                for g in range(NSUBB):
                    gsum = spool.tile([128, G], f32, tag="gsum", name="gsum")
                    nc.vector.tensor_add(gsum[:], T[g]["m1"][:], T[g]["m2"][:])
                    T[g]["gsum"] = gsum
                for g in range(NSUBB):
                    gs8 = spool.tile([128, 8], f32, tag="gs8", name="gs8")
                    nc.vector.max(gs8[:], T[g]["gsum"][:])
                    T[g]["gs8"] = gs8
                for g in range(NSUBB):
                    pen = spool.tile([128, G], f32, tag="pen", name="pen")
                    nc.vector.tensor_scalar(pen[:], T[g]["gsum"][:],
                                            T[g]["gs8"][:, 3:4],
                                            -1e30, op0=OP.is_lt, op1=OP.mult)
                    T[g]["pen"] = pen
                for g in range(NSUBB):
                    s_mask = mpool.tile([128, E], f32, tag="s_mask",
                                        name="s_mask")
                    pen_b = T[g]["pen"][:].unsqueeze(2).broadcast_to([128, G, GS])
                    nc.vector.tensor_tensor(
                        s_mask[:].rearrange("p (g s) -> p g s", g=G),
                        T[g]["s_choice"][:].rearrange("p (g s) -> p g s", g=G),
                        pen_b, op=OP.add)
                    T[g]["s_mask"] = s_mask
                for g in range(NSUBB):
                    v8 = spool.tile([128, 8], f32, tag="v8", name="v8")
                    nc.vector.max(v8[:], T[g]["s_mask"][:])
                    T[g]["v8"] = v8
                for g in range(NSUBB):
                    i8 = spool.tile([128, 8], u32, tag="i8", name="i8")
                    nc.vector.max_index(i8[:], T[g]["v8"][:], T[g]["s_mask"][:])
                    tok0 = tbase + g * 128
                    nc.gpsimd.dma_start(idx_out[tok0:tok0 + 128, :], i8[:])
                    T[g]["i8"] = i8
                for g in range(NSUBB):
                    r_sel = mpool.tile([128, E], f32, tag="r_sel", name="r_sel")
                    nc.vector.scalar_tensor_tensor(
                        r_sel[:], in0=T[g]["s_mask"][:], scalar=T[g]["v8"][:, 7:8],
                        in1=T[g]["s_raw"][:], op0=OP.is_ge, op1=OP.mult)
                    T[g]["r_sel"] = r_sel
                for g in range(NSUBB):
                    w8d = spool.tile([128, 8], f32, tag="w8d", name="w8d")
                    nc.vector.max(w8d[:], T[g]["r_sel"][:])
                    T[g]["w8d"] = w8d
                for g in range(NSUBB):
                    ri8 = spool.tile([128, 8], u32, tag="ri8", name="ri8")
                    nc.vector.max_index(ri8[:], T[g]["w8d"][:], T[g]["r_sel"][:])
                    T[g]["ri8"] = ri8
                for g in range(NSUBB):
                    eq64 = spool.tile([128, 8, 8], f32, tag="eq64", name="eq64")
                    i8_b = T[g]["i8"][:].unsqueeze(2).broadcast_to([128, 8, 8])
                    ri8_b = T[g]["ri8"][:].unsqueeze(1).broadcast_to([128, 8, 8])
                    nc.vector.tensor_tensor(eq64[:], i8_b, ri8_b, op=OP.is_equal)
                    T[g]["eq64"] = eq64
                for g in range(NSUBB):
                    w64 = spool.tile([128, 8, 8], f32, tag="w64", name="w64")
                    w8d_b = T[g]["w8d"][:].unsqueeze(1).broadcast_to([128, 8, 8])
                    nc.vector.tensor_tensor(w64[:], T[g]["eq64"][:], w8d_b,
                                            op=OP.mult)
                    T[g]["w64"] = w64
                for g in range(NSUBB):
                    w8p = spool.tile([128, 8], f32, tag="w8p", name="w8p")
                    nc.vector.reduce_sum(w8p[:], T[g]["w64"][:], axis=AX.X)
                    T[g]["w8p"] = w8p
                for g in range(NSUBB):
                    sum8 = spool.tile([128, 1], f32, tag="sum8", name="sum8")
                    nc.vector.reduce_sum(sum8[:], T[g]["w8d"][:], axis=AX.X)
                    T[g]["sum8"] = sum8
                for g in range(NSUBB):
                    rcp = spool.tile([128, 1], f32, tag="rcp", name="rcp")
                    nc.vector.reciprocal(rcp[:], T[g]["sum8"][:])
                    T[g]["rcp"] = rcp
                for g in range(NSUBB):
                    wf = spool.tile([128, 8], f32, tag="wf", name="wf")
                    nc.vector.tensor_scalar(wf[:], T[g]["w8p"][:],
                                            T[g]["rcp"][:, 0:1],
                                            SCALING, op0=OP.mult, op1=OP.mult)
                    tok0 = tbase + g * 128
                    nc.gpsimd.dma_start(w_out[tok0:tok0 + 128, :], wf[:])

    nc.compile()
    return nc


def _get_built():
    global _BUILT
    if _BUILT is None:
        _BUILT = _build()
    return _BUILT


def _part(a, inner):
    # [H, inner] -> [128, NCH, inner] with element (p, c, i) = a[c*128+p, i]
    return np.ascontiguousarray(a.reshape(NCH, 128, inner).transpose(1, 0, 2))


def _prep_in_maps(hidden_states, weight, e_score_correction_bias):
    f8 = ml_dtypes.float8_e4m3
    x = np.asarray(hidden_states, dtype=np.float32).reshape(T_FULL, H)
    xT = np.ascontiguousarray(x.T)                      # [H, T]
    x1f = xT.astype(np.float16)
    dx = xT - x1f.astype(np.float32)

    x1s = (x1f.astype(np.float32) * S_X1).astype(np.float16)   # exact scale
    dx8f = (dx * S_DX).astype(f8)
    xc8f = (xT * S_XC).astype(f8)

    W = np.asarray(weight, dtype=np.float32)
    Wt = np.ascontiguousarray(W.T)                      # [H, E]
    W1f = Wt.astype(np.float16)
    dW = Wt - W1f.astype(np.float32)
    w1h = _part((W1f.astype(np.float32) * S_W1).astype(np.float16), E)
    w18h = _part((W1f.astype(np.float32) * S_W1_8).astype(f8), E)
    dw8h = _part((dW * S_DW).astype(f8), E)
    w1h = np.ascontiguousarray(w1h.reshape(128, WF))
    w18h = np.ascontiguousarray(w18h.reshape(128, WF))
    dw8h = np.ascontiguousarray(dw8h.reshape(128, WF))

    b = np.asarray(e_score_correction_bias, dtype=np.float32)
    bias_rep = np.ascontiguousarray(np.tile(b[None, :], (128, 1)))
    ident = np.eye(128, dtype=np.float32)

    def blocks(a):
        # [128, NCH, T_CORE] -> [NB, 128, NCH*TB]
        v = a.reshape(128, NCH, NB, TB)
        return np.ascontiguousarray(v.transpose(2, 0, 1, 3)).reshape(NB, 128, XF)

    in_maps = []
    for c in range(N_CORES):
        sl = slice(c * T_CORE, (c + 1) * T_CORE)
        in_maps.append({
            "x1": blocks(_part(x1s[:, sl], T_CORE)),
            "dx8": blocks(_part(dx8f[:, sl], T_CORE)),
            "xc8": blocks(_part(xc8f[:, sl], T_CORE)),
            "w1": w1h, "w18": w18h, "dw8": dw8h,
            "bias_rep": bias_rep, "id_in": ident,
        })
    return in_maps


def kernel(hidden_states: np.ndarray, weight: np.ndarray,
           e_score_correction_bias: np.ndarray):
    in_maps = _prep_in_maps(hidden_states, weight, e_score_correction_bias)
    nc = _get_built()
    res = run_bass_kernel_spmd(nc, in_maps, list(range(N_CORES)))

    idx = np.concatenate([r["idx_out"] for r in res.results], axis=0).astype(np.int32)
    w = np.concatenate([r["w_out"] for r in res.results], axis=0).astype(np.float32)
    return idx, w
